# revision 19
# baseline (speedup 1.0000x reference)
"""Trainium2 Bass kernel for the CustomGNN message-passing model.

Strategy (dst-sharded, 8 cores, fp8 edge phase, batched dma_gather):
  - Nodes padded to 50176, split into 8 shards of 6272 (49 windows of 128).
  - Per layer:
      P1a: own-shard projections from the SBUF-resident x shard (PE
           transpose + matmuls); Dx -> SBUF fp8, Ax -> SBUF bf16;
           [E|B] rows -> local HBM slice [6272, 256] fp8.
      AG:  AllGather of the per-shard [E|B] slices into the full gather
           table [50176, 256] fp8 (256-byte row per node).
      P2:  windows processed in groups of 4; per group two dma_gather
           calls (src<32768 / src>=32768 halves, int16 indices) pull all
           edge rows [Ex|Bx] into SBUF in one shot; per window:
           indicator EQ (batched), Dx broadcast via one-hot matmuls
           (PSUM-grouped adds), sigmoid, sigma*Bx, then a one-hot
           slot-indicator matmul scatter-adds [sigma | contrib] into a
           per-pair PSUM window accumulator -> num/den.
      P3:  h = Ax + num/(den+eps) -> bf16 Hs; BN stats via ones-matmul
           accumulation over windows + pad-node trajectory correction +
           tiny AllReduce; batched BN apply + ReLU + residual updates
           the x shard in place.
  - Final: segment mean-pool via batch-indicator matmul + AllReduce, then
    the MLP head computed redundantly on every core.
"""

import sys

if "/opt/trn_rl_repo" not in sys.path:
    sys.path.insert(0, "/opt/trn_rl_repo")

import numpy as np
import ml_dtypes

from concourse import bacc, bass, mybir, tile, library_config
from concourse.bass_utils import run_bass_kernel_spmd

F32 = mybir.dt.float32
BF16 = mybir.dt.bfloat16
F8 = mybir.dt.float8e4
I16 = mybir.dt.int16

NPF8 = ml_dtypes.float8_e4m3
NPBF16 = ml_dtypes.bfloat16

P = 128
D = 128
L = 3
G = 64
DIM_OUT = 10
NCORES = 8
N_REAL = 50000
SH = 6272               # shard size (nodes per core), 49 windows of 128
WPC = SH // P           # windows per core = 49
NPAD = SH * NCORES      # 50176
EPS_BN = 1e-5
EPS_AGG = 1e-6
SPLIT = 32768           # int16 gather-index split point
G_WIN = 4               # windows per gather group

_cache = {}


def _layout(tlo, thi):
    """Group/window tile layout. Returns (groups, lo_base, hi_base, sumt).

    groups: list of dicts {ws, gt0, gt1, lo0, nlo, hi0, nhi}
    """
    lo_base = [0] * WPC
    hi_base = [0] * WPC
    groups = []
    tb = 0
    for a in range(0, WPC, G_WIN):
        ws = list(range(a, min(a + G_WIN, WPC)))
        gt0 = tb
        lo0 = tb
        for w in ws:
            lo_base[w] = tb
            tb += tlo[w]
        nlo = tb - lo0
        hi0 = tb
        for w in ws:
            hi_base[w] = tb
            tb += thi[w]
        nhi = tb - hi0
        groups.append(dict(ws=ws, gt0=gt0, gt1=tb, lo0=lo0, nlo=nlo,
                           hi0=hi0, nhi=nhi))
    return groups, lo_base, hi_base, tb


def _pack_edges(src, dst):
    """Sort edges by dst, split per window into lo/hi src halves, pack.

    Returns (idx16, sct, indt, tlo, thi):
      idx16 [NCORES, 128, sumt*8] int16 — wrapped gather indices
      sct   [NCORES, 128, sumt] bf16    — slot per (lane, tile), -1 pad
      indt  [NCORES, 128, sumt*128] f8  — transposed slot indicator
      tlo/thi: per-window tile counts (max over cores)
    """
    order = np.argsort(dst, kind="stable")
    src_s = src[order].astype(np.int64)
    dst_s = dst[order].astype(np.int64)
    win = dst_s // P                      # global window id = WPC*c + w
    half = (src_s >= SPLIT).astype(np.int64)
    nw = NCORES * WPC
    key = win * 2 + half
    order2 = np.argsort(key, kind="stable")
    src2 = src_s[order2]
    dst2 = dst_s[order2]
    key2 = key[order2]
    counts = np.bincount(key2, minlength=nw * 2)
    tmat = ((counts + P - 1) // P).reshape(NCORES, WPC, 2)
    tlo = [int(t) for t in tmat[:, :, 0].max(axis=0)]
    thi = [int(t) for t in tmat[:, :, 1].max(axis=0)]

    groups, lo_base, hi_base, sumt = _layout(tlo, thi)
    lo_base = np.array(lo_base)
    hi_base = np.array(hi_base)

    starts = np.zeros(nw * 2 + 1, np.int64)
    np.cumsum(counts, out=starts[1:])
    rank = np.arange(len(src2)) - starts[key2]
    c = key2 // (2 * WPC)
    rem = key2 % (2 * WPC)
    w = rem // 2
    h = rem % 2
    base = np.where(h == 0, lo_base[w], hi_base[w])
    flat = (base + rank // P) * P + rank % P

    idxA = np.zeros((NCORES, sumt * P), np.int64)
    slotA = np.full((NCORES, sumt * P), -1.0, np.float32)
    idxA[c, flat] = np.where(h == 0, src2, src2 - SPLIT)
    slotA[c, flat] = (dst2 % P).astype(np.float32)

    idx16 = np.ascontiguousarray(
        idxA.astype(np.int16).reshape(NCORES, sumt * 8, 16).transpose(0, 2, 1))
    idx16 = np.ascontiguousarray(np.tile(idx16, (1, 8, 1)))  # replicate to 128
    sct = np.ascontiguousarray(
        slotA.astype(NPBF16).reshape(NCORES, sumt, P).transpose(0, 2, 1))
    m = np.arange(P, dtype=np.float32)
    indt = (slotA[:, None, :] == m[None, :, None]).astype(NPF8)
    return idx16, sct, np.ascontiguousarray(indt), tlo, thi


def _build(tlo, thi):
    """Build + compile the SPMD Bass program for per-window tile counts."""
    groups, lo_base, hi_base, sumt = _layout(tlo, thi)
    gtmax = max(g["gt1"] - g["gt0"] for g in groups)
    twmax = max(tlo[w] + thi[w] for w in range(WPC))

    nc = bacc.Bacc("TRN2", target_bir_lowering=False, debug=False,
                   num_devices=NCORES)

    # ---- I/O -------------------------------------------------------------
    xn0 = nc.dram_tensor("xn0", [SH, D], F32, kind="ExternalInput")
    wts = nc.dram_tensor("wts", [L * 4 * D, D], BF16, kind="ExternalInput")
    bias_eb = nc.dram_tensor("bias_eb", [P, L * 2 * D], F32, kind="ExternalInput")
    bias_da = nc.dram_tensor("bias_da", [P, L * 2 * D], F32, kind="ExternalInput")
    gb = nc.dram_tensor("gb", [1, L * 2 * D], F32, kind="ExternalInput")
    idx_t = nc.dram_tensor("idx16", [P, sumt * 8], I16, kind="ExternalInput")
    sct = nc.dram_tensor("sct", [P, sumt], BF16, kind="ExternalInput")
    indt = nc.dram_tensor("indt", [P, sumt * P], F8, kind="ExternalInput")
    bslotT = nc.dram_tensor("bslotT", [P, WPC], F32, kind="ExternalInput")
    iota128 = nc.dram_tensor("iota128", [P, P], BF16, kind="ExternalInput")
    iota64 = nc.dram_tensor("iota64", [P, G], F32, kind="ExternalInput")
    ident = nc.dram_tensor("ident", [P, P], F32, kind="ExternalInput")
    onescol = nc.dram_tensor("onescol", [P, 1], BF16, kind="ExternalInput")
    npad_t = nc.dram_tensor("npadv", [1, 1], F32, kind="ExternalInput")
    rcnt = nc.dram_tensor("rcnt", [G, 1], F32, kind="ExternalInput")
    pw1 = nc.dram_tensor("pw1", [D, D], F32, kind="ExternalInput")
    pb1b = nc.dram_tensor("pb1b", [G, D], F32, kind="ExternalInput")
    pw2 = nc.dram_tensor("pw2", [D, DIM_OUT], F32, kind="ExternalInput")
    pb2b = nc.dram_tensor("pb2b", [G, DIM_OUT], F32, kind="ExternalInput")
    out_t = nc.dram_tensor("out", [G, DIM_OUT], F32, kind="ExternalOutput")

    # ---- internal DRAM ---------------------------------------------------
    eb_tabs = [
        nc.dram_tensor(f"eb_tab{l}", [NPAD, 2 * D], F8, addr_space="Shared")
        for l in range(L)
    ]
    ein = [nc.dram_tensor(f"ein{l}", [SH, 2 * D], F8) for l in range(L)]
    stin = [nc.dram_tensor(f"stin{l}", [1, 2 * D], F32) for l in range(L)]
    stout = [
        nc.dram_tensor(f"stout{l}", [1, 2 * D], F32, addr_space="Shared")
        for l in range(L)
    ]
    plin = nc.dram_tensor("plin", [G, D], F32)
    plout = nc.dram_tensor("plout", [G, D], F32, addr_space="Shared")
    scshd = [nc.dram_tensor(f"scshd{l}", [1, 2 * D], F32) for l in range(L)]

    RG = [list(range(NCORES))]
    ADD = mybir.AluOpType.add
    MUL = mybir.AluOpType.mult
    SUB = mybir.AluOpType.subtract
    EQ = mybir.AluOpType.is_equal
    AF = mybir.ActivationFunctionType

    with tile.TileContext(nc) as tc:
        ctxs = []

        def pool(**kw):
            p_ = tc.tile_pool(**kw)
            ctxs.append(p_)
            return p_.__enter__()

        persist = pool(name="persist", bufs=1)
        p_sel = pool(name="p_sel", bufs=3)
        p_tI = pool(name="p_tI", bufs=3)
        p_sig = pool(name="p_sig", bufs=2)
        p_indw = pool(name="p_indw", bufs=2)
        p_stage = pool(name="p_stage", bufs=2)
        p_t = pool(name="p_t", bufs=2)
        p_small = pool(name="p_small", bufs=2)
        p_tiny = pool(name="p_tiny", bufs=1)
        psT = pool(name="psT", bufs=1, space="PSUM")
        psProj = pool(name="psProj", bufs=2, space="PSUM")
        psPdx = pool(name="psPdx", bufs=2, space="PSUM")
        psWin = pool(name="psWin", bufs=2, space="PSUM")
        psMisc = pool(name="psMisc", bufs=1, space="PSUM")

        # ---- persistent SBUF --------------------------------------------
        xN = persist.tile([P, WPC * P], F32)       # x shard, node-major
        Ax = persist.tile([P, WPC * P], BF16)
        Dxb = persist.tile([P, WPC * P], F8)
        Hs = persist.tile([P, WPC * 2 * D], BF16)  # [h | h^2] per window
        idxsb = persist.tile([P, sumt * 8], I16)
        scsb = persist.tile([P, sumt], BF16)
        Wfb = persist.tile([P, L * 4 * D], BF16)
        bEB = persist.tile([P, L * 2 * D], F32)
        bDA = persist.tile([P, L * 2 * D], F32)
        gbsb = persist.tile([1, L * 2 * D], F32)
        io128 = persist.tile([P, P], BF16)
        io64 = persist.tile([P, G], F32)
        idsb = persist.tile([P, P], F32)
        onescolsb = persist.tile([P, 1], BF16)
        bslotsb = persist.tile([P, WPC], F32)
        npadsb = persist.tile([1, 1], F32)
        rcntsb = persist.tile([G, 1], F32)
        pw1sb = persist.tile([P, D], F32)
        pb1sb = persist.tile([G, D], F32)
        pw2sb = persist.tile([P, DIM_OUT], F32)
        pb2sb = persist.tile([G, DIM_OUT], F32)
        xpadR = persist.tile([1, D], F32)          # pad-node x row
        xpadC = persist.tile([P, 1], BF16)         # same, as a column

        nc.gpsimd.load_library(library_config.mlp)

        nc.sync.dma_start(out=xN[:].rearrange("p (w d) -> p w d", d=D),
                          in_=xn0[:].rearrange("(w p) d -> p w d", p=P))
        nc.sync.dma_start(out=idxsb[:], in_=idx_t[:, :])
        nc.sync.dma_start(out=scsb[:], in_=sct[:, :])
        nc.sync.dma_start(out=Wfb[:].rearrange("p (k d) -> p k d", d=D),
                          in_=wts[:].rearrange("(k p) d -> p k d", p=P))
        nc.sync.dma_start(out=bEB[:], in_=bias_eb[:, :])
        nc.sync.dma_start(out=bDA[:], in_=bias_da[:, :])
        nc.sync.dma_start(out=gbsb[:], in_=gb[:, :])
        nc.sync.dma_start(out=io128[:], in_=iota128[:, :])
        nc.sync.dma_start(out=io64[:], in_=iota64[:, :])
        nc.sync.dma_start(out=idsb[:], in_=ident[:, :])
        nc.sync.dma_start(out=onescolsb[:], in_=onescol[:, :])
        nc.sync.dma_start(out=bslotsb[:], in_=bslotT[:, :])
        nc.sync.dma_start(out=npadsb[:], in_=npad_t[:, :])
        nc.sync.dma_start(out=rcntsb[:], in_=rcnt[:, :])
        nc.sync.dma_start(out=pw1sb[:], in_=pw1[:, :])
        nc.sync.dma_start(out=pb1sb[:], in_=pb1b[:, :])
        nc.sync.dma_start(out=pw2sb[:], in_=pw2[:, :])
        nc.sync.dma_start(out=pb2sb[:], in_=pb2b[:, :])
        nc.vector.memset(xpadR[:], 0.0)
        nc.vector.memset(xpadC[:], 0.0)

        STG = 4

        for l in range(L):
            eb_tab = eb_tabs[l]

            # ---- P1a: own-shard D/A + E/B projections -------------------
            for w0 in range(0, WPC, STG):
                nwn = min(STG, WPC - w0)
                ebs = p_stage.tile([P, STG * 2 * D], F8, tag="ebs")
                for q in range(nwn):
                    w = w0 + q
                    pt = psT.tile([P, P], F32, tag="pt")
                    nc.tensor.transpose(
                        out=pt[:], in_=xN[:, w * P:(w + 1) * P],
                        identity=idsb[:])
                    xTb = p_small.tile([P, P], BF16, tag="xTb")
                    nc.scalar.copy(out=xTb[:], in_=pt[:])
                    pda = psProj.tile([P, 4 * D], F32, tag="proj")
                    nc.tensor.matmul(
                        out=pda[:], lhsT=xTb[:],
                        rhs=Wfb[:, l * 4 * D:(l + 1) * 4 * D],
                        start=True, stop=True)
                    nc.vector.tensor_tensor(
                        out=Dxb[:, w * P:(w + 1) * P], in0=pda[:, 0:D],
                        in1=bDA[:, l * 2 * D:l * 2 * D + D], op=ADD)
                    nc.vector.tensor_tensor(
                        out=Ax[:, w * P:(w + 1) * P], in0=pda[:, D:2 * D],
                        in1=bDA[:, l * 2 * D + D:(l + 1) * 2 * D], op=ADD)
                    nc.vector.tensor_tensor(
                        out=ebs[:, q * 2 * D:(q + 1) * 2 * D],
                        in0=pda[:, 2 * D:4 * D],
                        in1=bEB[:, l * 2 * D:(l + 1) * 2 * D], op=ADD)
                base = w0 * P
                dst_rows = ein[l][base:base + nwn * P, :].rearrange(
                    "(q p) d -> p q d", p=P)
                nc.sync.dma_start(
                    out=dst_rows,
                    in_=ebs[:, 0:nwn * 2 * D].rearrange(
                        "p (q d) -> p q d", d=2 * D))

            # pad-node trajectory: hpad = xpad @ W_A + biasA (pre-BN)
            ptp = psProj.tile([P, 4 * D], F32, tag="proj")
            nc.tensor.matmul(
                out=ptp[0:1, 0:D], lhsT=xpadC[:],
                rhs=Wfb[:, l * 4 * D + D:l * 4 * D + 2 * D],
                start=True, stop=True)
            hpadR = p_tiny.tile([1, 2 * D], F32, tag="hpad")
            nc.vector.tensor_tensor(
                out=hpadR[0:1, 0:D], in0=ptp[0:1, 0:D],
                in1=bDA[0:1, l * 2 * D + D:(l + 1) * 2 * D], op=ADD)
            nc.vector.tensor_tensor(
                out=hpadR[0:1, D:2 * D], in0=hpadR[0:1, 0:D],
                in1=hpadR[0:1, 0:D], op=MUL)

            # ---- AllGather E/B table -----------------------------------
            nc.gpsimd.collective_compute(
                "AllGather", mybir.AluOpType.bypass, replica_groups=RG,
                ins=[ein[l][:, :]], outs=[eb_tab[:, :]])

            # ---- P2: gather + scatter, grouped windows ------------------
            psStat = psMisc.tile([P, 512], F32, tag="stat")
            pwin = None
            for g in groups:
                gt0 = g["gt0"]
                ntile = g["gt1"] - gt0
                selg = p_sel.tile([P, gtmax * 2 * D], F8, tag="selg")
                # dma_gather is limited to 1024 indices (8 tiles) per call
                for (bt, nt, tab_view) in (
                        (g["lo0"], g["nlo"], eb_tab[0:SPLIT, :]),
                        (g["hi0"], g["nhi"], eb_tab[SPLIT:NPAD, :])):
                    for c0 in range(0, nt, 8):
                        cn = min(8, nt - c0)
                        t0 = bt + c0
                        o0 = (t0 - gt0) * 2 * D
                        nc.gpsimd.dma_gather(
                            out_ap=selg[:, o0:o0 + cn * 2 * D].rearrange(
                                "p (t e) -> p t e", e=2 * D),
                            in_ap=tab_view,
                            idxs_ap=idxsb[:, t0 * 8:(t0 + cn) * 8],
                            num_idxs=cn * P, num_idxs_reg=cn * P,
                            elem_size=2 * D)
                segmax = max(max(tlo), max(thi))
                for w in g["ws"]:
                    tw = tlo[w] + thi[w]
                    sigC = p_sig.tile([P, twmax * 2 * D], BF16, tag="sigC")
                    indwW = p_indw.tile([P, twmax * P], BF16, tag="indw")
                    segs = [(0, tlo[w], lo_base[w]), (tlo[w], thi[w], hi_base[w])]
                    for (sb, T, gt) in segs:
                        if T == 0:
                            continue
                        so = gt - gt0
                        tIseg = p_tI.tile([P, segmax * P], F8, tag="tIG")
                        nc.sync.dma_start(out=tIseg[:, 0:T * P],
                                          in_=indt[:, gt * P:(gt + T) * P])
                        nc.vector.tensor_tensor(
                            out=indwW[:, sb * P:(sb + T) * P].rearrange(
                                "p (t m) -> p t m", m=P),
                            in0=scsb[:, gt:gt + T].unsqueeze(2).to_broadcast(
                                [P, T, P]),
                            in1=io128[:].unsqueeze(1).to_broadcast([P, T, P]),
                            op=EQ)
                        for q0 in range(0, T, 4):
                            nq = min(4, T - q0)
                            psd = psPdx.tile([P, 512], F32, tag="pdx")
                            for j in range(nq):
                                nc.tensor.matmul(
                                    out=psd[:, j * P:(j + 1) * P],
                                    lhsT=tIseg[:, (q0 + j) * P:
                                               (q0 + j + 1) * P],
                                    rhs=Dxb[:, w * P:(w + 1) * P],
                                    start=True, stop=True)
                            nc.vector.tensor_tensor(
                                out=sigC[:, (sb + q0) * 2 * D:
                                         (sb + q0 + nq) * 2 * D].rearrange(
                                    "p (t e) -> p t e", e=2 * D)[:, :, 0:D],
                                in0=psd[:, 0:nq * P].rearrange(
                                    "p (t m) -> p t m", m=P),
                                in1=selg[:, (so + q0) * 2 * D:
                                         (so + q0 + nq) * 2 * D].rearrange(
                                    "p (t e) -> p t e", e=2 * D)[:, :, 0:D],
                                op=ADD)
                    sv = sigC[:, 0:tw * 2 * D].rearrange(
                        "p (t e) -> p t e", e=2 * D)
                    nc.scalar.activation(
                        out=sv[:, 0:tw, 0:D], in_=sv[:, 0:tw, 0:D],
                        func=AF.Sigmoid)
                    for (sb, T, gt) in segs:
                        if T == 0:
                            continue
                        so = gt - gt0
                        nc.vector.tensor_tensor(
                            out=sigC[:, sb * 2 * D:(sb + T) * 2 * D].rearrange(
                                "p (t e) -> p t e", e=2 * D)[:, :, D:2 * D],
                            in0=sigC[:, sb * 2 * D:(sb + T) * 2 * D].rearrange(
                                "p (t e) -> p t e", e=2 * D)[:, :, 0:D],
                            in1=selg[:, so * 2 * D:(so + T) * 2 * D].rearrange(
                                "p (t e) -> p t e", e=2 * D)[:, :, D:2 * D],
                            op=MUL)
                    # scatter into per-pair window accumulator
                    wl = w % 2
                    if wl == 0:
                        pwin = psWin.tile([P, 512], F32, tag="win")
                    for t in range(tw):
                        nc.tensor.matmul(
                            out=pwin[:, wl * 2 * D:(wl + 1) * 2 * D],
                            lhsT=indwW[:, t * P:(t + 1) * P],
                            rhs=sigC[:, t * 2 * D:(t + 1) * 2 * D],
                            start=(t == 0), stop=(t == tw - 1))
                    if wl == 1 or w == WPC - 1:
                        nwp = wl + 1
                        w0p = w - wl
                        den = p_t.tile([P, 2 * D], F32, tag="den")
                        psv = pwin[:, 0:nwp * 2 * D].rearrange(
                            "p (k e) -> p k e", e=2 * D)
                        dv = den[:, 0:nwp * D].rearrange(
                            "p (k d) -> p k d", d=D)
                        nc.vector.tensor_scalar_add(
                            out=dv, in0=psv[:, :, 0:D], scalar1=EPS_AGG)
                        nc.vector.reciprocal(out=den[:, 0:nwp * D],
                                             in_=den[:, 0:nwp * D])
                        hsv = Hs[:, w0p * 2 * D:(w0p + nwp) * 2 * D].rearrange(
                            "p (k e) -> p k e", e=2 * D)[:, :, 0:D]
                        nc.vector.tensor_tensor(
                            out=hsv, in0=psv[:, :, D:2 * D], in1=dv, op=MUL)
                        nc.vector.tensor_tensor(
                            out=hsv, in0=hsv,
                            in1=Ax[:, w0p * P:(w0p + nwp) * P].rearrange(
                                "p (k d) -> p k d", d=D), op=ADD)
                        for k in range(nwp):
                            ww = w0p + k
                            nc.scalar.activation(
                                out=Hs[:, ww * 2 * D + D:(ww + 1) * 2 * D],
                                in_=Hs[:, ww * 2 * D:ww * 2 * D + D],
                                func=AF.Square)
                            nc.tensor.matmul(
                                out=psStat[0:1, 0:2 * D], lhsT=onescolsb[:],
                                rhs=Hs[:, ww * 2 * D:(ww + 1) * 2 * D],
                                start=(ww == 0), stop=(ww == WPC - 1))

            # ---- P3: BN stats correction + allreduce + apply ------------
            statR = p_tiny.tile([1, 2 * D], F32, tag="statR")
            nc.vector.tensor_scalar(
                out=statR[:], in0=hpadR[:], scalar1=npadsb[0:1, 0:1],
                scalar2=None, op0=MUL)
            nc.vector.tensor_tensor(out=statR[:], in0=psStat[0:1, 0:2 * D],
                                    in1=statR[:], op=SUB)
            nc.sync.dma_start(out=stin[l][:, :], in_=statR[:])
            nc.gpsimd.collective_compute(
                "AllReduce", ADD, replica_groups=RG,
                ins=[stin[l][:, :]], outs=[stout[l][:, :]])
            ssum = p_tiny.tile([1, 2 * D], F32, tag="ssum")
            nc.sync.dma_start(out=ssum[:], in_=stout[l][:, :])

            scsh = p_tiny.tile([1, 2 * D], F32, tag="scsh")
            mu = p_tiny.tile([1, D], F32, tag="mu")
            tvar = p_tiny.tile([1, D], F32, tag="tvar")
            nc.vector.tensor_scalar_mul(out=mu[:], in0=ssum[0:1, 0:D],
                                        scalar1=1.0 / N_REAL)
            nc.vector.tensor_scalar_mul(out=tvar[:], in0=ssum[0:1, D:2 * D],
                                        scalar1=1.0 / N_REAL)
            musq = p_tiny.tile([1, D], F32, tag="musq")
            nc.vector.tensor_tensor(out=musq[:], in0=mu[:], in1=mu[:], op=MUL)
            nc.vector.tensor_tensor(out=tvar[:], in0=tvar[:], in1=musq[:],
                                    op=SUB)
            nc.vector.tensor_scalar_add(out=tvar[:], in0=tvar[:],
                                        scalar1=EPS_BN)
            nc.scalar.activation(out=tvar[:], in_=tvar[:], func=AF.Sqrt)
            nc.vector.reciprocal(out=tvar[:], in_=tvar[:])
            nc.vector.tensor_tensor(
                out=scsh[0:1, 0:D], in0=tvar[:],
                in1=gbsb[0:1, l * 2 * D:l * 2 * D + D], op=MUL)
            nc.vector.tensor_tensor(out=musq[:], in0=mu[:],
                                    in1=scsh[0:1, 0:D], op=MUL)
            nc.vector.tensor_tensor(
                out=scsh[0:1, D:2 * D],
                in0=gbsb[0:1, l * 2 * D + D:(l + 1) * 2 * D], in1=musq[:],
                op=SUB)
            scb = p_small.tile([P, 2 * D], F32, tag="scb")
            nc.sync.dma_start(out=scshd[l][:, :], in_=scsh[:])
            nc.sync.dma_start(
                out=scb[:],
                in_=scshd[l][0:1, :].to_broadcast([P, 2 * D]))

            # pad-node x update: xpad += relu(hpad*scale + shift)
            hpbn = p_tiny.tile([1, D], F32, tag="hpbn")
            nc.vector.tensor_tensor(out=hpbn[:], in0=hpadR[0:1, 0:D],
                                    in1=scsh[0:1, 0:D], op=MUL)
            nc.vector.tensor_tensor(out=hpbn[:], in0=hpbn[:],
                                    in1=scsh[0:1, D:2 * D], op=ADD)
            nc.scalar.activation(out=hpbn[:], in_=hpbn[:], func=AF.Relu)
            nc.vector.tensor_tensor(out=xpadR[:], in0=xpadR[:], in1=hpbn[:],
                                    op=ADD)
            ptx = psT.tile([P, P], F32, tag="pt")
            nc.tensor.transpose(out=ptx[:, 0:1], in_=xpadR[0:1, :],
                                identity=idsb[0:1, 0:1])
            nc.scalar.copy(out=xpadC[:], in_=ptx[:, 0:1])

            last = l == L - 1
            if last:
                ppool = psProj.tile([P, 2 * D], F32, tag="proj")
            for w0 in range(0, WPC, STG):
                nwn = min(STG, WPC - w0)
                hv = p_t.tile([P, STG * D], F32, tag="hv")
                hvv = hv[:, 0:nwn * D].rearrange("p (q d) -> p q d", d=D)
                hsv = Hs[:, w0 * 2 * D:(w0 + nwn) * 2 * D].rearrange(
                    "p (q e) -> p q e", e=2 * D)[:, :, 0:D]
                nc.vector.tensor_tensor(
                    out=hvv, in0=hsv,
                    in1=scb[:, 0:D].unsqueeze(1).to_broadcast([P, nwn, D]),
                    op=MUL)
                nc.vector.tensor_tensor(
                    out=hvv, in0=hvv,
                    in1=scb[:, D:2 * D].unsqueeze(1).to_broadcast([P, nwn, D]),
                    op=ADD)
                nc.scalar.activation(out=hv[:, 0:nwn * D], in_=hv[:, 0:nwn * D],
                                     func=AF.Relu)
                nc.vector.tensor_tensor(
                    out=xN[:, w0 * P:(w0 + nwn) * P],
                    in0=xN[:, w0 * P:(w0 + nwn) * P], in1=hv[:, 0:nwn * D],
                    op=ADD)
                if last:
                    for q in range(nwn):
                        w = w0 + q
                        ind64 = p_t.tile([P, G], F32, tag="ind64")
                        nc.vector.tensor_scalar(
                            out=ind64[:], in0=io64[:],
                            scalar1=bslotsb[:, w:w + 1], scalar2=None, op0=EQ)
                        nc.tensor.matmul(
                            out=ppool[0:G, 0:D], lhsT=ind64[:],
                            rhs=xN[:, w * P:(w + 1) * P],
                            start=(w == 0), stop=(w == WPC - 1))

        # ---- readout: pooled mean + MLP ---------------------------------
        pls = p_small.tile([G, D], F32, tag="pls")
        nc.vector.tensor_copy(out=pls[:], in_=ppool[0:G, 0:D])
        nc.sync.dma_start(out=plin[:, :], in_=pls[:])
        nc.gpsimd.collective_compute(
            "AllReduce", ADD, replica_groups=RG,
            ins=[plin[:, :]], outs=[plout[:, :]])
        pl2 = p_small.tile([G, D], F32, tag="pl2")
        nc.sync.dma_start(out=pl2[:], in_=plout[:, :])
        pooled = p_small.tile([G, D], F32, tag="pooled")
        nc.vector.tensor_scalar(out=pooled[:], in0=pl2[:, 0:D],
                                scalar1=rcntsb[:, 0:1], scalar2=None, op0=MUL)
        ptp = psT.tile([P, P], F32, tag="pt")
        nc.tensor.transpose(out=ptp[:, 0:G], in_=pooled[:],
                            identity=idsb[0:G, 0:G])
        pooledT = p_small.tile([P, G], F32, tag="pooledT")
        nc.scalar.copy(out=pooledT[:], in_=ptp[:, 0:G])
        ph1 = psProj.tile([P, 2 * D], F32, tag="proj")
        nc.tensor.matmul(out=ph1[0:G, 0:D], lhsT=pooledT[:], rhs=pw1sb[:],
                         start=True, stop=True)
        h1 = p_small.tile([G, D], F32, tag="h1")
        nc.vector.tensor_tensor(out=h1[:], in0=ph1[0:G, 0:D], in1=pb1sb[:],
                                op=ADD)
        nc.scalar.activation(out=h1[:], in_=h1[:], func=AF.Relu)
        pth = psT.tile([P, P], F32, tag="pt")
        nc.tensor.transpose(out=pth[:, 0:G], in_=h1[:],
                            identity=idsb[0:G, 0:G])
        h1T = p_small.tile([P, G], F32, tag="h1T")
        nc.scalar.copy(out=h1T[:], in_=pth[:, 0:G])
        pout = psProj.tile([P, 2 * D], F32, tag="proj")
        nc.tensor.matmul(out=pout[0:G, 0:DIM_OUT], lhsT=h1T[:], rhs=pw2sb[:],
                         start=True, stop=True)
        osb = p_small.tile([G, DIM_OUT], F32, tag="osb")
        nc.vector.tensor_tensor(out=osb[:], in0=pout[0:G, 0:DIM_OUT],
                                in1=pb2sb[:], op=ADD)
        nc.sync.dma_start(out=out_t[:, :], in_=osb[:])

        for p_ in reversed(ctxs):
            p_.__exit__(None, None, None)

    nc.compile()
    return nc


def _prepare_inputs(x, edge_index, batch_vec, lin_w, lin_b, bn_gamma, bn_beta,
                    post_w1, post_b1, post_w2, post_b2):
    x = np.asarray(x, np.float32)
    ei = np.asarray(edge_index)
    bv = np.asarray(batch_vec).astype(np.int64)
    lin_w = np.asarray(lin_w, np.float32)
    lin_b = np.asarray(lin_b, np.float32)

    idx16, sct, indt, tlo, thi = _pack_edges(ei[0].astype(np.int64),
                                             ei[1].astype(np.int64))

    x0 = np.zeros((NPAD, D), np.float32)
    x0[:N_REAL] = x

    # wts rows per layer: [D (k=2), A (k=0), E (k=3), B (k=1)]
    wts = np.concatenate(
        [np.stack([lin_w[l, 2], lin_w[l, 0], lin_w[l, 3], lin_w[l, 1]])
         for l in range(L)]
    ).reshape(L * 4 * D, D).astype(NPBF16)

    bias_eb = np.zeros((P, L * 2 * D), np.float32)
    bias_da = np.zeros((P, L * 2 * D), np.float32)
    for l in range(L):
        bias_eb[:, l * 2 * D:l * 2 * D + D] = lin_b[l, 3][None, :]
        bias_eb[:, l * 2 * D + D:(l + 1) * 2 * D] = lin_b[l, 1][None, :]
        bias_da[:, l * 2 * D:l * 2 * D + D] = lin_b[l, 2][None, :]
        bias_da[:, l * 2 * D + D:(l + 1) * 2 * D] = lin_b[l, 0][None, :]
    gbv = np.zeros((1, L * 2 * D), np.float32)
    for l in range(L):
        gbv[0, l * 2 * D:l * 2 * D + D] = np.asarray(bn_gamma, np.float32)[l]
        gbv[0, l * 2 * D + D:(l + 1) * 2 * D] = np.asarray(bn_beta,
                                                           np.float32)[l]

    bslotv = np.full(NPAD, -1.0, np.float32)
    bslotv[:N_REAL] = bv.astype(np.float32)

    shared = dict(
        wts=wts, bias_eb=bias_eb, bias_da=bias_da,
        gb=gbv,
        iota128=np.tile(np.arange(P, dtype=NPBF16)[None, :], (P, 1)),
        iota64=np.tile(np.arange(G, dtype=np.float32)[None, :], (P, 1)),
        ident=np.eye(P, dtype=np.float32),
        onescol=np.ones((P, 1), NPBF16),
        pw1=np.asarray(post_w1, np.float32),
        pb1b=np.tile(np.asarray(post_b1, np.float32)[None, :], (G, 1)),
        pw2=np.asarray(post_w2, np.float32),
        rcnt=(1.0 / np.maximum(
            np.bincount(bv, minlength=G), 1.0)).astype(
                np.float32).reshape(G, 1),
        pb2b=np.tile(np.asarray(post_b2, np.float32)[None, :], (G, 1)),
    )
    in_maps = []
    for c in range(NCORES):
        lo = c * SH
        m = dict(shared)
        m["xn0"] = np.ascontiguousarray(x0[lo:lo + SH])
        m["idx16"] = np.ascontiguousarray(idx16[c])
        m["sct"] = np.ascontiguousarray(sct[c])
        m["indt"] = np.ascontiguousarray(indt[c])
        m["bslotT"] = np.ascontiguousarray(
            bslotv[lo:lo + SH].reshape(WPC, P).T)
        m["npadv"] = np.array(
            [[float(NPAD - N_REAL) if c == NCORES - 1 else 0.0]], np.float32)
        in_maps.append(m)
    return in_maps, tlo, thi


def kernel(x, edge_attr, edge_index, batch_vec, lin_w, lin_b, bn_gamma,
           bn_beta, post_w1, post_b1, post_w2, post_b2, **_unused):
    in_maps, tlo, thi = _prepare_inputs(
        x, edge_index, batch_vec, lin_w, lin_b, bn_gamma, bn_beta,
        post_w1, post_b1, post_w2, post_b2)
    key = (tuple(tlo), tuple(thi))
    if key not in _cache:
        _cache[key] = _build(tlo, thi)
    nc = _cache[key]
    res = run_bass_kernel_spmd(nc, in_maps, core_ids=list(range(NCORES)))
    return np.asarray(res.results[0]["out"], np.float32)
